# revision 49
# baseline (speedup 1.0000x reference)
"""MoE block (B=16,N=1024,C=768,E=8,H=192,D=4,K=2) on 8 NeuronCores.

Strategy: data-parallel over B (2 samples/core). Per sample, noisy gating in
fp16 (fp32 PSUM), top-2 experts, one indirect-DMA gather of each chosen
expert's packed fp8 weights, then the 2-layer MLP entirely in fp8 DoubleRow
matmuls (2 contraction rows/partition, fp32 accumulate), exact Gelu on the
scalar engine, gate scaling fused into the h activations, channel-major fp16
output with the residual added from the fp16 x kept in SBUF. The [C, N]
output layout is untransposed on the host.

Host prep (pure value-preserving reshape/quantize): x shipped once as fp16
and once as fp8 in [128, 6, 1024] partition-major transposed layout; gate_w
gathered by task_id to fp16; fc1/fc2 weights packed per-expert into one fp8
row-block (x8 scale on fc1, x4 on fc2, undone on device) so one gather per
expert fetches everything incl. biases.
"""
import numpy as np
import ml_dtypes

import concourse.bass as bass
import concourse.mybir as mybir
import concourse.tile as tile
from concourse import bacc
from concourse.bass_utils import run_bass_kernel_spmd

bf16 = ml_dtypes.bfloat16
f16 = np.float16
f8 = ml_dtypes.float8_e4m3fn
f32 = np.float32
AF = mybir.ActivationFunctionType
ALU = mybir.AluOpType
DR = mybir.MatmulPerfMode.DoubleRow
dt = mybir.dt

B, N, C = 16, 1024, 768
E, H, D, TOPK = 8, 192, 4, 2
NCORES = 8
SPC = B // NCORES          # samples per core = 2
C_K = C // 128             # 6 chunks over channels
TCH = N // 128             # 8 token chunks
W1S, W2S = 8.0, 4.0        # fp8 weight scales (undone via act scale / gates)
# packed per-expert fp8 row layout (one indirect gather per expert):
# [0:1152)    fc1: k-chunk j at cols 192j..192j+192, row p = 8*W1[128j+p, h]
# [1152:1920) fc2 head: col 1152+c, row p = 4*W2[h=p, c]
# [1920:2688) fc2 tail: col 1920+c, row p<64 = 4*W2[h=128+p, c]; row 64 = 4*b2
# [2688:2690) fc1 bias: col 2688 row p = b1[p]; col 2689 row p<64 = b1[128+p]
PCK = 2690

_cache = {}


def _build(reps=1, general_bias=False):
    key = ("nc", reps, general_bias)
    if key in _cache:
        return _cache[key]
    nc = bacc.Bacc("TRN2", target_bir_lowering=False, debug=False,
                   num_devices=NCORES)

    x16_d = nc.dram_tensor("x16", [SPC, 128, C_K, N], dt.float16, kind="ExternalInput").ap()
    x8_d = nc.dram_tensor("x8", [SPC, 128, C_K, N], dt.float8e4, kind="ExternalInput").ap()
    gw_d = nc.dram_tensor("gw16", [128, SPC, C_K, 2 * E], dt.float16, kind="ExternalInput").ap()
    ep_d = nc.dram_tensor("eps_r", [128, SPC, TCH, E], dt.float32, kind="ExternalInput").ap()
    wp_d = nc.dram_tensor("wpack", [TOPK * E * 128, PCK], dt.float8e4, kind="ExternalInput").ap()
    id_d = nc.dram_tensor("id16", [128, 128], dt.float16, kind="ExternalInput").ap()
    y_d = nc.dram_tensor("y", [SPC, 128, C_K, N], dt.float16, kind="ExternalOutput").ap()

    with tile.TileContext(nc) as tc:
        with tc.tile_pool(name="const", bufs=1) as cp, \
             tc.tile_pool(name="x16", bufs=2) as x16p, \
             tc.tile_pool(name="x8", bufs=2) as x8p, \
             tc.tile_pool(name="gin", bufs=2) as ginp, \
             tc.tile_pool(name="gate", bufs=2) as gp, \
             tc.tile_pool(name="wt", bufs=4) as wtp, \
             tc.tile_pool(name="h8", bufs=4) as h8p, \
             tc.tile_pool(name="g16", bufs=4) as g16p, \
             tc.tile_pool(name="ys", bufs=2) as ysp, \
             tc.tile_pool(name="ps_g", bufs=2, space="PSUM") as pgp, \
             tc.tile_pool(name="ps_t", bufs=1, space="PSUM") as ptp, \
             tc.tile_pool(name="ps_1", bufs=2, space="PSUM") as ps1p, \
             tc.tile_pool(name="ps_2", bufs=3, space="PSUM") as ps2p:

            # constants
            iota_i = cp.tile([128, 1], dt.int32, tag="iota_i")
            iota_f = cp.tile([128, 1], dt.float32, tag="iota_f")
            nc.gpsimd.iota(iota_i[:], pattern=[[0, 1]], base=0, channel_multiplier=1)
            nc.vector.tensor_copy(iota_f[:], iota_i[:])
            ones_r = cp.tile([1, 128], dt.float32, tag="ones_r")
            nc.vector.memset(ones_r[:], 1.0)
            ones_c = cp.tile([128, 1], dt.float32, tag="ones_c")
            nc.vector.memset(ones_c[:], 1.0)
            id16 = cp.tile([128, 128], dt.float16, tag="id16")
            iota_r = cp.tile([1, 128], dt.int32, tag="iota_r")
            nc.gpsimd.iota(iota_r[:], pattern=[[1, 128]], base=0,
                           channel_multiplier=0)
            iota_rf = cp.tile([1, 128], dt.float32, tag="iota_rf")
            nc.vector.tensor_copy(iota_rf[:], iota_r[:])
            ones_t2 = cp.tile([1, TOPK], dt.float32, tag="ones_t2")
            nc.vector.memset(ones_t2[:], 1.0)
            warm = cp.tile([1, 2], dt.float32, tag="warm")
            nc.scalar.activation(warm[:], ones_t2[:], AF.Gelu)
            ones_sq = cp.tile([128, 128], dt.float32, tag="ones_sq")
            nc.vector.memset(ones_sq[:, :], 1.0)
            iota2_i = cp.tile([128, TOPK], dt.int32, tag="iota2_i")
            nc.gpsimd.iota(iota2_i[:], pattern=[[E * 128, TOPK]], base=0,
                           channel_multiplier=1)
            iota2_f = cp.tile([128, TOPK], dt.float32, tag="iota2_f")
            nc.vector.tensor_copy(iota2_f[:], iota2_i[:])

            for rep in range(reps):
              # ---- A. issue loads ordered for the earliest critical path:
              # sample 0's gating inputs, then its fc1 input, then sample 1.
              x16t = [x16p.tile([128, C_K, N], dt.float16, tag=f"x16_{s}",
                                name=f"x16_{s}") for s in range(SPC)]
              x8t = [x8p.tile([128, C_K, N], dt.float8e4, tag=f"x8_{s}",
                              name=f"x8_{s}") for s in range(SPC)]
              gwt = ginp.tile([128, SPC, C_K, 2 * E], dt.float16, tag="gw")
              epst = ginp.tile([128, SPC, TCH, E], dt.float32, tag="ep")
              nc.sync.dma_start(gwt[:, :, :, :], gw_d[:, :, :, :])
              nc.sync.dma_start(x16t[0][:, :, 0:512], x16_d[0, :, :, 0:512])
              nc.sync.dma_start(x16t[0][:, :, 512:1024], x16_d[0, :, :, 512:1024])
              nc.sync.dma_start(epst[:, :, :, :], ep_d[:, :, :, :])
              nc.sync.dma_start(x16t[1][:, :, 0:512], x16_d[1, :, :, 0:512])
              nc.sync.dma_start(x16t[1][:, :, 512:1024], x16_d[1, :, :, 512:1024])
              nc.sync.dma_start(x8t[0][:, :, :], x8_d[0, :, :, :])
              nc.sync.dma_start(id16[:, :], id_d[:, :])
              nc.sync.dma_start(x8t[1][:, :, :], x8_d[1, :, :, :])

              # h8 pad memsets up front while Pool is idle
              h8tiles = [[h8p.tile([128, 2, N], dt.float8e4, tag=f"h8_{s}_{j}",
                                   name=f"h8_{s}_{j}") for j in range(TOPK)]
                         for s in range(SPC)]
              for s in range(SPC):
                  for j in range(TOPK):
                      nc.gpsimd.memset(h8tiles[s][j][64:128, 1, :], 0.0)
                      if general_bias:
                          # fc2 bias: 0.25 * (4*g_j*b2 row) = g_j*b2
                          nc.gpsimd.memset(h8tiles[s][j][64:65, 1, :], 0.25)

              # ---- B. gating per sample. K=2 gates are the constants
              # softmax([1, 0]) up to O(1e-6/gap); only top-2 indices are
              # computed. softplus = relu(v) + poly(min(|v|,6)) evaluated
              # Estrin-style on DVE (max err 5e-5), so the only ACT table
              # ever loaded is Gelu's.
              PC = [0.7030958864859523, -0.4991347018389747,
                    0.12139956534475345, 0.006388911044793425,
                    -0.01108461419835834, 0.002966883877695811,
                    -0.0004000833569692521, 2.827203585505132e-05,
                    -8.329831435070043e-07]  # c0 includes the +0.01

              def gating_front(s):
                  """pg matmuls + softplus/noise reduction -> ewsp [128, E]"""
                  gs = gp.tile([128, TCH, 2 * E], dt.float32, tag=f"gs{s}",
                               name=f"gs{s}")
                  for r in range(TCH // 2):
                      pg = pgp.tile([128, 2, 2 * E], dt.float32, space="PSUM",
                                    tag="pg", name="pg")
                      for half in range(2):
                          t = 2 * r + half
                          for k in range(C_K):
                              nc.tensor.matmul(
                                  out=pg[:, half, :],
                                  lhsT=x16t[s][:, k, 128 * t:128 * (t + 1)],
                                  rhs=gwt[:, s, k, :],
                                  start=(half == 0 and k == 0),
                                  stop=(half == 1 and k == C_K - 1),
                                  skip_group_check=True)
                      nc.vector.tensor_copy(gs[:, 2 * r:2 * r + 2, :], pg[:, :, :])
                  vn = gs[:, :, E:2 * E]
                  av = gp.tile([128, TCH, E], dt.float32, tag="av", name="av")
                  nc.scalar.activation(av[:, :, :], vn, AF.Abs)
                  rl = gp.tile([128, TCH, E], dt.float32, tag="rl", name="rl")
                  nc.scalar.activation(rl[:, :, :], vn, AF.Relu)
                  w = gp.tile([128, TCH, E], dt.float32, tag="w", name="w")
                  nc.vector.tensor_scalar(out=w[:, :, :], in0=av[:, :, :],
                                          scalar1=6.0, scalar2=None, op0=ALU.min)
                  qt = [gp.tile([128, TCH, E], dt.float32, tag=f"q{i}",
                                name=f"q{i}") for i in range(4)]
                  for i in range(4):
                      nc.vector.tensor_scalar(
                          out=qt[i][:, :, :], in0=w[:, :, :],
                          scalar1=PC[2 * i + 1], scalar2=PC[2 * i],
                          op0=ALU.mult, op1=ALU.add)
                  w2 = gp.tile([128, TCH, E], dt.float32, tag="w2", name="w2")
                  nc.vector.tensor_tensor(out=w2[:, :, :], in0=w[:, :, :],
                                          in1=w[:, :, :], op=ALU.mult)
                  w4 = gp.tile([128, TCH, E], dt.float32, tag="w4", name="w4")
                  nc.vector.tensor_tensor(out=w4[:, :, :], in0=w2[:, :, :],
                                          in1=w2[:, :, :], op=ALU.mult)
                  r0 = gp.tile([128, TCH, E], dt.float32, tag="r0", name="r0")
                  nc.vector.tensor_tensor(out=r0[:, :, :], in0=qt[1][:, :, :],
                                          in1=w2[:, :, :], op=ALU.mult)
                  nc.vector.tensor_add(r0[:, :, :], r0[:, :, :], qt[0][:, :, :])
                  hi = gp.tile([128, TCH, E], dt.float32, tag="hi", name="hi")
                  nc.vector.tensor_scalar(out=hi[:, :, :], in0=w2[:, :, :],
                                          scalar1=PC[8], scalar2=None,
                                          op0=ALU.mult)
                  nc.vector.tensor_add(hi[:, :, :], hi[:, :, :], qt[3][:, :, :])
                  nc.vector.tensor_tensor(out=hi[:, :, :], in0=hi[:, :, :],
                                          in1=w2[:, :, :], op=ALU.mult)
                  nc.vector.tensor_add(hi[:, :, :], hi[:, :, :], qt[2][:, :, :])
                  nc.vector.tensor_tensor(out=hi[:, :, :], in0=hi[:, :, :],
                                          in1=w4[:, :, :], op=ALU.mult)
                  nc.vector.tensor_add(r0[:, :, :], r0[:, :, :], hi[:, :, :])
                  nc.vector.tensor_add(r0[:, :, :], r0[:, :, :], rl[:, :, :])
                  prod = gp.tile([128, TCH, E], dt.float32, tag="prod",
                                 name="prod")
                  nc.vector.tensor_tensor(out=prod[:, :, :], in0=r0[:, :, :],
                                          in1=epst[:, s, :, :], op=ALU.mult)
                  redp = gp.tile([128, E], dt.float32, tag="redp", name="redp")
                  nc.vector.tensor_reduce(
                      out=redp[:, :],
                      in_=prod[:, :, :].rearrange("p t e -> p e t"),
                      axis=mybir.AxisListType.X, op=ALU.add)
                  redc = gp.tile([128, E], dt.float32, tag="redc", name="redc")
                  nc.vector.tensor_reduce(
                      out=redc[:, :],
                      in_=gs[:, :, 0:E].rearrange("p t e -> p e t"),
                      axis=mybir.AxisListType.X, op=ALU.add)
                  ewsp = gp.tile([128, E], dt.float32, tag="ewsp", name="ewsp")
                  nc.vector.tensor_add(ewsp[:, :], redp[:, :], redc[:, :])
                  return ewsp

              def gating_top(s, ewsp):
                  """replicated partition-sum via all-ones matmul, top-2,
                  per-partition gather offsets, weight gathers"""
                  ews_ps = ptp.tile([128, E], dt.float32, space="PSUM",
                                    tag="pt", name="ews_ps")
                  nc.tensor.matmul(out=ews_ps[:, :], lhsT=ones_sq[:, :],
                                   rhs=ewsp[:, :], start=True, stop=True)
                  ewsb = gp.tile([128, E], dt.float32, tag="ewsb", name="ewsb")
                  nc.vector.tensor_copy(ewsb[:], ews_ps[:])
                  mx = gp.tile([128, E], dt.float32, tag="mx", name="mx")
                  mi = gp.tile([128, E], dt.uint32, tag="mi", name="mi")
                  nc.vector.max_with_indices(mx[:], mi[:], ewsb[:, :])
                  mif = gp.tile([128, TOPK], dt.float32, tag="mif", name="mif")
                  nc.vector.tensor_copy(mif[:], mi[:, 0:TOPK])
                  b2f = gp.tile([128, TOPK], dt.float32, tag="b2f", name="b2f")
                  nc.vector.tensor_scalar(out=b2f[:], in0=mif[:], scalar1=128.0,
                                          scalar2=None, op0=ALU.mult)
                  nc.vector.tensor_add(b2f[:], b2f[:], iota2_f[:])
                  gi2 = gp.tile([128, TOPK], dt.uint32, tag="gi2", name="gi2")
                  nc.vector.tensor_copy(gi2[:], b2f[:])
                  wts = []
                  for j in range(TOPK):
                      wt = wtp.tile([128, PCK], dt.float8e4, tag=f"wt{s}_{j}",
                                    name=f"wt{s}_{j}")
                      nc.gpsimd.indirect_dma_start(
                          out=wt[:], out_offset=None, in_=wp_d[:],
                          in_offset=bass.IndirectOffsetOnAxis(ap=gi2[:, j:j + 1],
                                                              axis=0))
                      wts.append(wt)
                  return wts

              # ---- C. experts: fc1 DoubleRow + gelu + gate scaling ----
              GATES = (0.7310585786300049, 0.2689414213699951)  # softmax([1,0])

              def experts(s, wts):
                  h8s = []
                  for j in range(TOPK):
                      wt = wts[j]
                      w1v = wt[:, 0:6 * H].rearrange("p (k h) -> p k h", k=C_K)
                      h8 = h8tiles[s][j]
                      for m in range(2):
                          msz = 128 if m == 0 else H - 128
                          for n in range(2):
                              ps1 = ps1p.tile([msz, 512], dt.float32,
                                              space="PSUM", tag="ps1",
                                              name="ps1")
                              for jp in range(C_K // 2):
                                  nc.tensor.matmul(
                                      out=ps1[:, :],
                                      lhsT=w1v[:, 2 * jp:2 * jp + 2,
                                               128 * m:128 * m + msz],
                                      rhs=x8t[s][:, 2 * jp:2 * jp + 2,
                                                 512 * n:512 * (n + 1)],
                                      start=(jp == 0), stop=(jp == C_K // 2 - 1),
                                      perf_mode=DR)
                              tgt = (h8[:, 0, 512 * n:512 * (n + 1)] if m == 0
                                     else h8[0:msz, 1, 512 * n:512 * (n + 1)])
                              nc.scalar.activation(
                                  tgt, ps1[:, :], AF.Gelu,
                                  bias=wt[0:msz, 2688 + m:2689 + m],
                                  scale=1.0 / W1S)
                      h8s.append(h8)
                  return h8s

              ews0 = gating_front(0)
              wts0 = gating_top(0, ews0)
              with tc.high_priority():
                  ews1 = gating_front(1)
                  wts1 = gating_top(1, ews1)
              h8s0 = experts(0, wts0)
              with tc.high_priority():
                  h8s1 = experts(1, wts1)
              hstates = [(wts0, h8s0), (wts1, h8s1)]

              # ---- D. fc2 DoubleRow + residual + store ----
              for s in range(SPC):
                  wts, h8s = hstates[s]
                  ys = ysp.tile([128, C_K, N], dt.float16, tag="ys")
                  w2v = [wt[:, 6 * H:6 * H + 2 * C].rearrange("p (g c) -> p g c", g=2)
                         for wt in wts]
                  for cc in range(C_K):
                      for n in range(2):
                          ps2 = ps2p.tile([128, 512], dt.float32, space="PSUM",
                                          tag="ps2")
                          for j in range(TOPK):
                              nc.tensor.matmul(
                                  out=ps2[:, :],
                                  lhsT=w2v[j][:, :, 128 * cc:128 * (cc + 1)],
                                  rhs=h8s[j][:, :, 512 * n:512 * (n + 1)],
                                  start=(j == 0),
                                  stop=(j == TOPK - 1
                                        and (s * 12 + cc * 2 + n) % 3 != 1),
                                  perf_mode=DR)
                          blk = s * 12 + cc * 2 + n
                          if blk % 3 != 1:
                              # residual on DVE straight from PSUM (x16 is 4x)
                              nc.vector.tensor_tensor(
                                  out=ys[:, cc, 512 * n:512 * (n + 1)],
                                  in0=ps2[:, :],
                                  in1=x16t[s][:, cc, 512 * n:512 * (n + 1)],
                                  op=ALU.add)
                          else:
                              # residual via identity matmul, copy on ACT
                              nc.tensor.matmul(
                                  out=ps2[:, :], lhsT=id16[:, :],
                                  rhs=x16t[s][:, cc, 512 * n:512 * (n + 1)],
                                  start=False, stop=True, skip_group_check=True)
                              nc.scalar.activation(
                                  ys[:, cc, 512 * n:512 * (n + 1)], ps2[:, :],
                                  AF.Copy)
                      if cc % 2 == 1:
                          nc.sync.dma_start(y_d[s, :, cc - 1:cc + 1, :],
                                            ys[:, cc - 1:cc + 1, :])

    nc.compile()
    _cache[key] = nc
    return nc


def _prep_inputs(x, task_ids, eps, gate_w, fc1_w, fc1_b, fc2_w, fc2_b):
    x = np.asarray(x, dtype=f32)
    task_ids = np.asarray(task_ids).astype(np.int64)
    eps = np.asarray(eps, dtype=f32)
    gate_w = np.asarray(gate_w, dtype=f32)
    fc1_w = np.asarray(fc1_w, dtype=f32)
    fc1_b = np.asarray(fc1_b, dtype=f32)
    fc2_w = np.asarray(fc2_w, dtype=f32)
    fc2_b = np.asarray(fc2_b, dtype=f32)

    # x transposed to [B, 128, 6, 1024]: partition p holds channels 128j+p
    xT = np.ascontiguousarray(
        x.transpose(0, 2, 1).reshape(B, C_K, 128, N).transpose(0, 2, 1, 3))
    x16 = (4.0 * xT).astype(f16)   # 4x: lets the fc2 combine stay scale-free
    x8 = xT.astype(f8)

    gw = gate_w[task_ids]                                  # [B, C, 2E]
    # [ncore, 128, SPC, C_K, 2E]
    gw16 = np.ascontiguousarray(
        0.25 * gw.reshape(NCORES, SPC, C_K, 128, 2 * E).transpose(0, 3, 1, 2, 4)
    ).astype(f16)

    # [ncore, 128, SPC, TCH, E]
    eps_r = np.ascontiguousarray(
        eps.reshape(NCORES, SPC, TCH, 128, E).transpose(0, 3, 1, 2, 4))

    w1T = fc1_w.transpose(0, 2, 1)                         # [E, C, H]
    w2T = fc2_w.transpose(0, 2, 1)                         # [E, H, C]
    GATES = (0.7310585786300049, 0.2689414213699951)       # softmax([1, 0])
    wpack = np.zeros((TOPK, E, 128, PCK), dtype=f32)
    for j in range(C_K):
        wpack[:, :, :, H * j:H * (j + 1)] = \
            W1S * w1T[None, :, 128 * j:128 * (j + 1), :]
    for r in range(TOPK):
        g4 = W2S * GATES[r]
        wpack[r, :, :, 1152:1920] = g4 * w2T[:, 0:128, :]
        wpack[r, :, 0:64, 1920:2688] = g4 * w2T[:, 128:H, :]
        wpack[r, :, 64, 1920:2688] = g4 * fc2_b
    wpack[:, :, :, 2688] = fc1_b[None, :, 0:128]
    wpack[:, :, 0:64, 2689] = fc1_b[None, :, 128:H]
    wpack = wpack.reshape(TOPK * E * 128, PCK).astype(f8)
    id16 = np.eye(128, dtype=f16)

    general_bias = bool(np.any(fc2_b))

    in_maps = []
    for c in range(NCORES):
        sl = slice(SPC * c, SPC * (c + 1))
        in_maps.append({
            "x16": x16[sl], "x8": x8[sl], "gw16": gw16[c],
            "eps_r": eps_r[c], "wpack": wpack, "id16": id16,
        })
    return in_maps, general_bias


def kernel(x, task_ids, eps, gate_w, fc1_w, fc1_b, fc2_w, fc2_b, _trace=False):
    in_maps, general_bias = _prep_inputs(
        x, task_ids, eps, gate_w, fc1_w, fc1_b, fc2_w, fc2_b)
    nc = _build(general_bias=general_bias)
    res = run_bass_kernel_spmd(nc, in_maps, list(range(NCORES)), trace=_trace)
    y = np.concatenate([res.results[c]["y"] for c in range(NCORES)], axis=0)
    kernel.last_results = res
    # [B, 128, 6, 1024] -> [B, N, C] with c = 128j + p
    out = 0.25 * y.astype(np.float32).transpose(0, 3, 2, 1).reshape(B, N, C)
    return np.ascontiguousarray(out)


# revision 50
# speedup vs baseline: 1.0089x; 1.0089x over previous
"""MoE block (B=16,N=1024,C=768,E=8,H=192,D=4,K=2) on 8 NeuronCores.

Strategy: data-parallel over B (2 samples/core). Per sample, noisy gating in
fp16 (fp32 PSUM), top-2 experts, one indirect-DMA gather of each chosen
expert's packed fp8 weights, then the 2-layer MLP entirely in fp8 DoubleRow
matmuls (2 contraction rows/partition, fp32 accumulate), exact Gelu on the
scalar engine, gate scaling fused into the h activations, channel-major fp16
output with the residual added from the fp16 x kept in SBUF. The [C, N]
output layout is untransposed on the host.

Host prep (pure value-preserving reshape/quantize): x shipped once as fp16
and once as fp8 in [128, 6, 1024] partition-major transposed layout; gate_w
gathered by task_id to fp16; fc1/fc2 weights packed per-expert into one fp8
row-block (x8 scale on fc1, x4 on fc2, undone on device) so one gather per
expert fetches everything incl. biases.
"""
import numpy as np
import ml_dtypes

import concourse.bass as bass
import concourse.mybir as mybir
import concourse.tile as tile
from concourse import bacc
from concourse.bass_utils import run_bass_kernel_spmd

bf16 = ml_dtypes.bfloat16
f16 = np.float16
f8 = ml_dtypes.float8_e4m3fn
f32 = np.float32
AF = mybir.ActivationFunctionType
ALU = mybir.AluOpType
DR = mybir.MatmulPerfMode.DoubleRow
dt = mybir.dt

B, N, C = 16, 1024, 768
E, H, D, TOPK = 8, 192, 4, 2
NCORES = 8
SPC = B // NCORES          # samples per core = 2
C_K = C // 128             # 6 chunks over channels
TCH = N // 128             # 8 token chunks
W1S, W2S = 8.0, 4.0        # fp8 weight scales (undone via act scale / gates)
# packed per-expert fp8 row layout (one indirect gather per expert):
# [0:1152)    fc1: k-chunk j at cols 192j..192j+192, row p = 8*W1[128j+p, h]
# [1152:1920) fc2 head: col 1152+c, row p = 4*W2[h=p, c]
# [1920:2688) fc2 tail: col 1920+c, row p<64 = 4*W2[h=128+p, c]; row 64 = 4*b2
# [2688:2690) fc1 bias: col 2688 row p = b1[p]; col 2689 row p<64 = b1[128+p]
PCK = 2690

_cache = {}


def _build(reps=1, general_bias=False):
    key = ("nc", reps, general_bias)
    if key in _cache:
        return _cache[key]
    nc = bacc.Bacc("TRN2", target_bir_lowering=False, debug=False,
                   num_devices=NCORES)

    x16_d = nc.dram_tensor("x16", [SPC, 128, C_K, N], dt.float16, kind="ExternalInput").ap()
    x8_d = nc.dram_tensor("x8", [SPC, 128, C_K, N], dt.float8e4, kind="ExternalInput").ap()
    gw_d = nc.dram_tensor("gw16", [128, SPC, C_K, 2 * E], dt.float16, kind="ExternalInput").ap()
    ep_d = nc.dram_tensor("eps_r", [128, SPC, TCH, E], dt.float32, kind="ExternalInput").ap()
    wp_d = nc.dram_tensor("wpack", [TOPK * E * 128, PCK], dt.float8e4, kind="ExternalInput").ap()
    id_d = nc.dram_tensor("id16", [128, 128], dt.float16, kind="ExternalInput").ap()
    y_d = nc.dram_tensor("y", [SPC, 128, C_K, N], dt.float16, kind="ExternalOutput").ap()

    with tile.TileContext(nc) as tc:
        with tc.tile_pool(name="const", bufs=1) as cp, \
             tc.tile_pool(name="x16", bufs=2) as x16p, \
             tc.tile_pool(name="x8", bufs=2) as x8p, \
             tc.tile_pool(name="gin", bufs=2) as ginp, \
             tc.tile_pool(name="gate", bufs=2) as gp, \
             tc.tile_pool(name="wt", bufs=4) as wtp, \
             tc.tile_pool(name="h8", bufs=4) as h8p, \
             tc.tile_pool(name="g16", bufs=4) as g16p, \
             tc.tile_pool(name="ys", bufs=2) as ysp, \
             tc.tile_pool(name="ps_g", bufs=2, space="PSUM") as pgp, \
             tc.tile_pool(name="ps_t", bufs=1, space="PSUM") as ptp, \
             tc.tile_pool(name="ps_1", bufs=2, space="PSUM") as ps1p, \
             tc.tile_pool(name="ps_2", bufs=3, space="PSUM") as ps2p:

            # constants
            iota_i = cp.tile([128, 1], dt.int32, tag="iota_i")
            iota_f = cp.tile([128, 1], dt.float32, tag="iota_f")
            nc.gpsimd.iota(iota_i[:], pattern=[[0, 1]], base=0, channel_multiplier=1)
            nc.vector.tensor_copy(iota_f[:], iota_i[:])
            ones_r = cp.tile([1, 128], dt.float32, tag="ones_r")
            nc.vector.memset(ones_r[:], 1.0)
            ones_c = cp.tile([128, 1], dt.float32, tag="ones_c")
            nc.vector.memset(ones_c[:], 1.0)
            id16 = cp.tile([128, 128], dt.float16, tag="id16")
            iota_r = cp.tile([1, 128], dt.int32, tag="iota_r")
            nc.gpsimd.iota(iota_r[:], pattern=[[1, 128]], base=0,
                           channel_multiplier=0)
            iota_rf = cp.tile([1, 128], dt.float32, tag="iota_rf")
            nc.vector.tensor_copy(iota_rf[:], iota_r[:])
            ones_t2 = cp.tile([1, TOPK], dt.float32, tag="ones_t2")
            nc.vector.memset(ones_t2[:], 1.0)
            warm = cp.tile([1, 2], dt.float32, tag="warm")
            nc.scalar.activation(warm[:], ones_t2[:], AF.Gelu)
            ones_sq = cp.tile([128, 128], dt.float32, tag="ones_sq")
            nc.vector.memset(ones_sq[:, :], 1.0)
            iota2_i = cp.tile([128, TOPK], dt.int32, tag="iota2_i")
            nc.gpsimd.iota(iota2_i[:], pattern=[[E * 128, TOPK]], base=0,
                           channel_multiplier=1)
            iota2_f = cp.tile([128, TOPK], dt.float32, tag="iota2_f")
            nc.vector.tensor_copy(iota2_f[:], iota2_i[:])

            for rep in range(reps):
              # ---- A. issue loads ordered for the earliest critical path:
              # sample 0's gating inputs, then its fc1 input, then sample 1.
              x16t = [x16p.tile([128, C_K, N], dt.float16, tag=f"x16_{s}",
                                name=f"x16_{s}") for s in range(SPC)]
              x8t = [x8p.tile([128, C_K, N], dt.float8e4, tag=f"x8_{s}",
                              name=f"x8_{s}") for s in range(SPC)]
              gwt = ginp.tile([128, SPC, C_K, 2 * E], dt.float16, tag="gw")
              epst = ginp.tile([128, SPC, TCH, E], dt.float32, tag="ep")
              nc.sync.dma_start(x16t[0][:, :, 0:512], x16_d[0, :, :, 0:512])
              nc.sync.dma_start(gwt[:, :, :, :], gw_d[:, :, :, :])
              nc.sync.dma_start(x16t[0][:, :, 512:1024], x16_d[0, :, :, 512:1024])
              nc.sync.dma_start(epst[:, :, :, :], ep_d[:, :, :, :])
              nc.sync.dma_start(x16t[1][:, :, 0:512], x16_d[1, :, :, 0:512])
              nc.sync.dma_start(x16t[1][:, :, 512:1024], x16_d[1, :, :, 512:1024])
              nc.sync.dma_start(x8t[0][:, :, :], x8_d[0, :, :, :])
              nc.sync.dma_start(id16[:, :], id_d[:, :])
              nc.sync.dma_start(x8t[1][:, :, :], x8_d[1, :, :, :])

              # h8 pad memsets up front while Pool is idle
              h8tiles = [[h8p.tile([128, 2, N], dt.float8e4, tag=f"h8_{s}_{j}",
                                   name=f"h8_{s}_{j}") for j in range(TOPK)]
                         for s in range(SPC)]
              for s in range(SPC):
                  for j in range(TOPK):
                      nc.gpsimd.memset(h8tiles[s][j][64:128, 1, :], 0.0)
                      if general_bias:
                          # fc2 bias: 0.25 * (4*g_j*b2 row) = g_j*b2
                          nc.gpsimd.memset(h8tiles[s][j][64:65, 1, :], 0.25)

              # ---- B. gating per sample. K=2 gates are the constants
              # softmax([1, 0]) up to O(1e-6/gap); only top-2 indices are
              # computed. softplus = relu(v) + poly(min(|v|,6)) evaluated
              # Estrin-style on DVE (max err 5e-5), so the only ACT table
              # ever loaded is Gelu's.
              PC = [0.7030958864859523, -0.4991347018389747,
                    0.12139956534475345, 0.006388911044793425,
                    -0.01108461419835834, 0.002966883877695811,
                    -0.0004000833569692521, 2.827203585505132e-05,
                    -8.329831435070043e-07]  # c0 includes the +0.01

              def gating_front(s):
                  """pg matmuls + softplus/noise reduction -> ewsp [128, E]"""
                  gs = gp.tile([128, TCH, 2 * E], dt.float32, tag=f"gs{s}",
                               name=f"gs{s}")
                  for r in range(TCH // 2):
                      pg = pgp.tile([128, 2, 2 * E], dt.float32, space="PSUM",
                                    tag="pg", name="pg")
                      for half in range(2):
                          t = 2 * r + half
                          for k in range(C_K):
                              nc.tensor.matmul(
                                  out=pg[:, half, :],
                                  lhsT=x16t[s][:, k, 128 * t:128 * (t + 1)],
                                  rhs=gwt[:, s, k, :],
                                  start=(half == 0 and k == 0),
                                  stop=(half == 1 and k == C_K - 1),
                                  skip_group_check=True)
                      nc.vector.tensor_copy(gs[:, 2 * r:2 * r + 2, :], pg[:, :, :])
                  vn = gs[:, :, E:2 * E]
                  av = gp.tile([128, TCH, E], dt.float32, tag="av", name="av")
                  nc.scalar.activation(av[:, :, :], vn, AF.Abs)
                  rl = gp.tile([128, TCH, E], dt.float32, tag="rl", name="rl")
                  nc.scalar.activation(rl[:, :, :], vn, AF.Relu)
                  w = gp.tile([128, TCH, E], dt.float32, tag="w", name="w")
                  nc.vector.tensor_scalar(out=w[:, :, :], in0=av[:, :, :],
                                          scalar1=6.0, scalar2=None, op0=ALU.min)
                  qt = [gp.tile([128, TCH, E], dt.float32, tag=f"q{i}",
                                name=f"q{i}") for i in range(4)]
                  for i in range(4):
                      nc.vector.tensor_scalar(
                          out=qt[i][:, :, :], in0=w[:, :, :],
                          scalar1=PC[2 * i + 1], scalar2=PC[2 * i],
                          op0=ALU.mult, op1=ALU.add)
                  w2 = gp.tile([128, TCH, E], dt.float32, tag="w2", name="w2")
                  nc.vector.tensor_tensor(out=w2[:, :, :], in0=w[:, :, :],
                                          in1=w[:, :, :], op=ALU.mult)
                  w4 = gp.tile([128, TCH, E], dt.float32, tag="w4", name="w4")
                  nc.vector.tensor_tensor(out=w4[:, :, :], in0=w2[:, :, :],
                                          in1=w2[:, :, :], op=ALU.mult)
                  r0 = gp.tile([128, TCH, E], dt.float32, tag="r0", name="r0")
                  nc.vector.tensor_tensor(out=r0[:, :, :], in0=qt[1][:, :, :],
                                          in1=w2[:, :, :], op=ALU.mult)
                  nc.vector.tensor_add(r0[:, :, :], r0[:, :, :], qt[0][:, :, :])
                  hi = gp.tile([128, TCH, E], dt.float32, tag="hi", name="hi")
                  nc.vector.tensor_scalar(out=hi[:, :, :], in0=w2[:, :, :],
                                          scalar1=PC[8], scalar2=None,
                                          op0=ALU.mult)
                  nc.vector.tensor_add(hi[:, :, :], hi[:, :, :], qt[3][:, :, :])
                  nc.vector.tensor_tensor(out=hi[:, :, :], in0=hi[:, :, :],
                                          in1=w2[:, :, :], op=ALU.mult)
                  nc.vector.tensor_add(hi[:, :, :], hi[:, :, :], qt[2][:, :, :])
                  nc.vector.tensor_tensor(out=hi[:, :, :], in0=hi[:, :, :],
                                          in1=w4[:, :, :], op=ALU.mult)
                  nc.vector.tensor_add(r0[:, :, :], r0[:, :, :], hi[:, :, :])
                  nc.vector.tensor_add(r0[:, :, :], r0[:, :, :], rl[:, :, :])
                  prod = gp.tile([128, TCH, E], dt.float32, tag="prod",
                                 name="prod")
                  nc.vector.tensor_tensor(out=prod[:, :, :], in0=r0[:, :, :],
                                          in1=epst[:, s, :, :], op=ALU.mult)
                  redp = gp.tile([128, E], dt.float32, tag="redp", name="redp")
                  nc.vector.tensor_reduce(
                      out=redp[:, :],
                      in_=prod[:, :, :].rearrange("p t e -> p e t"),
                      axis=mybir.AxisListType.X, op=ALU.add)
                  redc = gp.tile([128, E], dt.float32, tag="redc", name="redc")
                  nc.vector.tensor_reduce(
                      out=redc[:, :],
                      in_=gs[:, :, 0:E].rearrange("p t e -> p e t"),
                      axis=mybir.AxisListType.X, op=ALU.add)
                  ewsp = gp.tile([128, E], dt.float32, tag="ewsp", name="ewsp")
                  nc.vector.tensor_add(ewsp[:, :], redp[:, :], redc[:, :])
                  return ewsp

              def gating_top(s, ewsp):
                  """replicated partition-sum via all-ones matmul, top-2,
                  per-partition gather offsets, weight gathers"""
                  ews_ps = ptp.tile([128, E], dt.float32, space="PSUM",
                                    tag="pt", name="ews_ps")
                  nc.tensor.matmul(out=ews_ps[:, :], lhsT=ones_sq[:, :],
                                   rhs=ewsp[:, :], start=True, stop=True)
                  ewsb = gp.tile([128, E], dt.float32, tag="ewsb", name="ewsb")
                  nc.vector.tensor_copy(ewsb[:], ews_ps[:])
                  mx = gp.tile([128, E], dt.float32, tag="mx", name="mx")
                  mi = gp.tile([128, E], dt.uint32, tag="mi", name="mi")
                  nc.vector.max_with_indices(mx[:], mi[:], ewsb[:, :])
                  mif = gp.tile([128, TOPK], dt.float32, tag="mif", name="mif")
                  nc.vector.tensor_copy(mif[:], mi[:, 0:TOPK])
                  b2f = gp.tile([128, TOPK], dt.float32, tag="b2f", name="b2f")
                  nc.vector.tensor_scalar(out=b2f[:], in0=mif[:], scalar1=128.0,
                                          scalar2=None, op0=ALU.mult)
                  nc.vector.tensor_add(b2f[:], b2f[:], iota2_f[:])
                  gi2 = gp.tile([128, TOPK], dt.uint32, tag="gi2", name="gi2")
                  nc.vector.tensor_copy(gi2[:], b2f[:])
                  wts = []
                  for j in range(TOPK):
                      wt = wtp.tile([128, PCK], dt.float8e4, tag=f"wt{s}_{j}",
                                    name=f"wt{s}_{j}")
                      nc.gpsimd.indirect_dma_start(
                          out=wt[:], out_offset=None, in_=wp_d[:],
                          in_offset=bass.IndirectOffsetOnAxis(ap=gi2[:, j:j + 1],
                                                              axis=0))
                      wts.append(wt)
                  return wts

              # ---- C. experts: fc1 DoubleRow + gelu + gate scaling ----
              GATES = (0.7310585786300049, 0.2689414213699951)  # softmax([1,0])

              def experts(s, wts):
                  h8s = []
                  for j in range(TOPK):
                      wt = wts[j]
                      w1v = wt[:, 0:6 * H].rearrange("p (k h) -> p k h", k=C_K)
                      h8 = h8tiles[s][j]
                      for m in range(2):
                          msz = 128 if m == 0 else H - 128
                          for n in range(2):
                              ps1 = ps1p.tile([msz, 512], dt.float32,
                                              space="PSUM", tag="ps1",
                                              name="ps1")
                              for jp in range(C_K // 2):
                                  nc.tensor.matmul(
                                      out=ps1[:, :],
                                      lhsT=w1v[:, 2 * jp:2 * jp + 2,
                                               128 * m:128 * m + msz],
                                      rhs=x8t[s][:, 2 * jp:2 * jp + 2,
                                                 512 * n:512 * (n + 1)],
                                      start=(jp == 0), stop=(jp == C_K // 2 - 1),
                                      perf_mode=DR)
                              tgt = (h8[:, 0, 512 * n:512 * (n + 1)] if m == 0
                                     else h8[0:msz, 1, 512 * n:512 * (n + 1)])
                              nc.scalar.activation(
                                  tgt, ps1[:, :], AF.Gelu,
                                  bias=wt[0:msz, 2688 + m:2689 + m],
                                  scale=1.0 / W1S)
                      h8s.append(h8)
                  return h8s

              ews0 = gating_front(0)
              wts0 = gating_top(0, ews0)
              with tc.high_priority():
                  ews1 = gating_front(1)
                  wts1 = gating_top(1, ews1)
              h8s0 = experts(0, wts0)
              with tc.high_priority():
                  h8s1 = experts(1, wts1)
              hstates = [(wts0, h8s0), (wts1, h8s1)]

              # ---- D. fc2 DoubleRow + residual + store ----
              for s in range(SPC):
                  wts, h8s = hstates[s]
                  ys = ysp.tile([128, C_K, N], dt.float16, tag="ys")
                  w2v = [wt[:, 6 * H:6 * H + 2 * C].rearrange("p (g c) -> p g c", g=2)
                         for wt in wts]
                  for cc in range(C_K):
                      for n in range(2):
                          ps2 = ps2p.tile([128, 512], dt.float32, space="PSUM",
                                          tag="ps2")
                          for j in range(TOPK):
                              nc.tensor.matmul(
                                  out=ps2[:, :],
                                  lhsT=w2v[j][:, :, 128 * cc:128 * (cc + 1)],
                                  rhs=h8s[j][:, :, 512 * n:512 * (n + 1)],
                                  start=(j == 0),
                                  stop=(j == TOPK - 1
                                        and (s * 12 + cc * 2 + n) % 3 != 1),
                                  perf_mode=DR)
                          blk = s * 12 + cc * 2 + n
                          if blk % 3 != 1:
                              # residual on DVE straight from PSUM (x16 is 4x)
                              nc.vector.tensor_tensor(
                                  out=ys[:, cc, 512 * n:512 * (n + 1)],
                                  in0=ps2[:, :],
                                  in1=x16t[s][:, cc, 512 * n:512 * (n + 1)],
                                  op=ALU.add)
                          else:
                              # residual via identity matmul, copy on ACT
                              nc.tensor.matmul(
                                  out=ps2[:, :], lhsT=id16[:, :],
                                  rhs=x16t[s][:, cc, 512 * n:512 * (n + 1)],
                                  start=False, stop=True, skip_group_check=True)
                              nc.scalar.activation(
                                  ys[:, cc, 512 * n:512 * (n + 1)], ps2[:, :],
                                  AF.Copy)
                      if cc % 2 == 1:
                          nc.sync.dma_start(y_d[s, :, cc - 1:cc + 1, :],
                                            ys[:, cc - 1:cc + 1, :])

    nc.compile()
    _cache[key] = nc
    return nc


def _prep_inputs(x, task_ids, eps, gate_w, fc1_w, fc1_b, fc2_w, fc2_b):
    x = np.asarray(x, dtype=f32)
    task_ids = np.asarray(task_ids).astype(np.int64)
    eps = np.asarray(eps, dtype=f32)
    gate_w = np.asarray(gate_w, dtype=f32)
    fc1_w = np.asarray(fc1_w, dtype=f32)
    fc1_b = np.asarray(fc1_b, dtype=f32)
    fc2_w = np.asarray(fc2_w, dtype=f32)
    fc2_b = np.asarray(fc2_b, dtype=f32)

    # x transposed to [B, 128, 6, 1024]: partition p holds channels 128j+p
    xT = np.ascontiguousarray(
        x.transpose(0, 2, 1).reshape(B, C_K, 128, N).transpose(0, 2, 1, 3))
    x16 = (4.0 * xT).astype(f16)   # 4x: lets the fc2 combine stay scale-free
    x8 = xT.astype(f8)

    gw = gate_w[task_ids]                                  # [B, C, 2E]
    # [ncore, 128, SPC, C_K, 2E]
    gw16 = np.ascontiguousarray(
        0.25 * gw.reshape(NCORES, SPC, C_K, 128, 2 * E).transpose(0, 3, 1, 2, 4)
    ).astype(f16)

    # [ncore, 128, SPC, TCH, E]
    eps_r = np.ascontiguousarray(
        eps.reshape(NCORES, SPC, TCH, 128, E).transpose(0, 3, 1, 2, 4))

    w1T = fc1_w.transpose(0, 2, 1)                         # [E, C, H]
    w2T = fc2_w.transpose(0, 2, 1)                         # [E, H, C]
    GATES = (0.7310585786300049, 0.2689414213699951)       # softmax([1, 0])
    wpack = np.zeros((TOPK, E, 128, PCK), dtype=f32)
    for j in range(C_K):
        wpack[:, :, :, H * j:H * (j + 1)] = \
            W1S * w1T[None, :, 128 * j:128 * (j + 1), :]
    for r in range(TOPK):
        g4 = W2S * GATES[r]
        wpack[r, :, :, 1152:1920] = g4 * w2T[:, 0:128, :]
        wpack[r, :, 0:64, 1920:2688] = g4 * w2T[:, 128:H, :]
        wpack[r, :, 64, 1920:2688] = g4 * fc2_b
    wpack[:, :, :, 2688] = fc1_b[None, :, 0:128]
    wpack[:, :, 0:64, 2689] = fc1_b[None, :, 128:H]
    wpack = wpack.reshape(TOPK * E * 128, PCK).astype(f8)
    id16 = np.eye(128, dtype=f16)

    general_bias = bool(np.any(fc2_b))

    in_maps = []
    for c in range(NCORES):
        sl = slice(SPC * c, SPC * (c + 1))
        in_maps.append({
            "x16": x16[sl], "x8": x8[sl], "gw16": gw16[c],
            "eps_r": eps_r[c], "wpack": wpack, "id16": id16,
        })
    return in_maps, general_bias


def kernel(x, task_ids, eps, gate_w, fc1_w, fc1_b, fc2_w, fc2_b, _trace=False):
    in_maps, general_bias = _prep_inputs(
        x, task_ids, eps, gate_w, fc1_w, fc1_b, fc2_w, fc2_b)
    nc = _build(general_bias=general_bias)
    res = run_bass_kernel_spmd(nc, in_maps, list(range(NCORES)), trace=_trace)
    y = np.concatenate([res.results[c]["y"] for c in range(NCORES)], axis=0)
    kernel.last_results = res
    # [B, 128, 6, 1024] -> [B, N, C] with c = 128j + p
    out = 0.25 * y.astype(np.float32).transpose(0, 3, 2, 1).reshape(B, N, C)
    return np.ascontiguousarray(out)
